# revision 10
# baseline (speedup 1.0000x reference)
"""Trainium2 Bass kernel for nn_CilLayer: [128,65536,3] f32 -> [128,65536,2] f32.

out0 = -90*(clip(x,-1,1)+1)
out1 = (180/pi)*atan2(z,y)  computed as  180*(z>=0) - 90 - (180/pi)*atan(y/z)

Sharding: batch dim split evenly across 8 NeuronCores (16 batches/core),
purely elementwise, no communication.
"""
import sys
import math

if '/opt/trn_rl_repo' not in sys.path:
    sys.path.insert(0, '/opt/trn_rl_repo')

import numpy as np

B, L = 128, 65536
NCORES = 8
BPC = B // NCORES            # batches per core
NPT = BPC * L                # points per core = 1,048,576
P = 128                      # SBUF partitions
FD = 1024                    # points per partition per tile
T = NPT // (P * FD)          # tiles per core
FACTOR = 180.0 / math.pi

_CACHE = {}


def _build():
    from concourse import mybir, tile, bacc
    f32 = mybir.dt.float32
    AFT = mybir.ActivationFunctionType
    ALU = mybir.AluOpType

    nc = bacc.Bacc("TRN2", debug=False)
    x = nc.dram_tensor("x", [NPT * 3], f32, kind="ExternalInput").ap()
    o = nc.dram_tensor("o", [NPT * 2], f32, kind="ExternalOutput").ap()
    xt = x.rearrange("(t p m) -> t p m", t=T, p=P)
    ot = o.rearrange("(t p m) -> t p m", t=T, p=P)

    with tile.TileContext(nc) as tc:
        with tc.tile_pool(name="io", bufs=3) as iop, \
             tc.tile_pool(name="tmp", bufs=2) as tp:
            for t in range(T):
                tin = iop.tile([P, 3 * FD], f32, tag="in")
                nc.sync.dma_start(tin[:], xt[t])
                v = tin[:].rearrange("p (f c) -> p f c", c=3)
                xv, yv, zv = v[:, :, 0], v[:, :, 1], v[:, :, 2]

                tout = iop.tile([P, 2 * FD], f32, tag="out")
                ov = tout[:].rearrange("p (f c) -> p f c", c=2)
                ov0, ov1 = ov[:, :, 0], ov[:, :, 1]

                # out1 = -FACTOR*(atan(y/z) - (pi/2)*sign(z))
                trc = tp.tile([P, FD], f32, tag="trc")
                nc.vector.reciprocal_approx_fast(trc[:], zv)
                tdiv = tp.tile([P, FD], f32, tag="tdiv")
                nc.vector.tensor_tensor(tdiv[:], yv, trc[:], ALU.mult)
                ta = tp.tile([P, FD], f32, tag="ta")
                nc.scalar.activation(ta[:], tdiv[:], AFT.Arctan)
                ts = tp.tile([P, FD], f32, tag="ts")
                nc.scalar.activation(ts[:], zv, AFT.Sign)
                tu = tp.tile([P, FD], f32, tag="tu")
                nc.vector.scalar_tensor_tensor(
                    tu[:], ts[:], -math.pi / 2.0, ta[:], ALU.mult, ALU.add)
                nc.scalar.activation(ov1, tu[:], AFT.Copy, scale=-FACTOR)

                # out0 = -90*clip(x,-1,1) - 90
                tclip = tp.tile([P, FD], f32, tag="tclip")
                nc.vector.tensor_scalar(
                    tclip[:], xv, 1.0, -1.0, ALU.min, ALU.max)
                nc.vector.tensor_scalar(
                    ov0, tclip[:], -90.0, -90.0, ALU.mult, ALU.add)

                nc.sync.dma_start(ot[t], tout[:])
    nc.compile()
    return nc


def _get_nc():
    if 'nc' not in _CACHE:
        _CACHE['nc'] = _build()
    return _CACHE['nc']


def kernel(inputs):
    from concourse import bass_utils
    inputs = np.ascontiguousarray(inputs, dtype=np.float32)
    assert inputs.shape == (B, L, 3), inputs.shape
    nc = _get_nc()
    in_maps = [
        {"x": inputs[c * BPC:(c + 1) * BPC].reshape(-1)} for c in range(NCORES)
    ]
    res = bass_utils.run_bass_kernel_spmd(nc, in_maps, list(range(NCORES)))
    out = np.concatenate(
        [res.results[c]["o"].reshape(BPC, L, 2) for c in range(NCORES)], axis=0)
    return out


# revision 12
# speedup vs baseline: 1.0358x; 1.0358x over previous
"""Trainium2 Bass kernel for nn_CilLayer: [128,65536,3] f32 -> [128,65536,2] f32.

out0 = -90*(clip(x,-1,1)+1)
out1 = (180/pi)*atan2(z,y)  computed as  180*(z>=0) - 90 - (180/pi)*atan(y/z)

Sharding: batch dim split evenly across 8 NeuronCores (16 batches/core),
purely elementwise, no communication.
"""
import sys
import math

if '/opt/trn_rl_repo' not in sys.path:
    sys.path.insert(0, '/opt/trn_rl_repo')

import numpy as np

B, L = 128, 65536
NCORES = 8
BPC = B // NCORES            # batches per core
NPT = BPC * L                # points per core = 1,048,576
P = 128                      # SBUF partitions
FD = 1024                    # points per partition per tile
T = NPT // (P * FD)          # tiles per core
FACTOR = 180.0 / math.pi

_CACHE = {}


def _build():
    from concourse import mybir, tile, bacc
    f32 = mybir.dt.float32
    AFT = mybir.ActivationFunctionType
    ALU = mybir.AluOpType

    nc = bacc.Bacc("TRN2", debug=False)
    x = nc.dram_tensor("x", [NPT * 3], f32, kind="ExternalInput").ap()
    o = nc.dram_tensor("o", [NPT * 2], f32, kind="ExternalOutput").ap()
    xt = x.rearrange("(t p m) -> t p m", t=T, p=P)
    ot = o.rearrange("(t p m) -> t p m", t=T, p=P)

    with tile.TileContext(nc) as tc:
        with tc.tile_pool(name="io", bufs=4) as iop, \
             tc.tile_pool(name="tmp", bufs=2) as tp:
            for t in range(T):
                tin = iop.tile([P, 3 * FD], f32, tag="in")
                nc.sync.dma_start(tin[:], xt[t])
                v = tin[:].rearrange("p (f c) -> p f c", c=3)
                xv, yv, zv = v[:, :, 0], v[:, :, 1], v[:, :, 2]

                tout = iop.tile([P, 2 * FD], f32, tag="out")
                ov = tout[:].rearrange("p (f c) -> p f c", c=2)
                ov0, ov1 = ov[:, :, 0], ov[:, :, 1]

                # out1 = -FACTOR*(atan(y/z) - (pi/2)*sign(z))
                trc = tp.tile([P, FD], f32, tag="trc")
                nc.vector.reciprocal_approx_fast(trc[:], zv)
                tdiv = tp.tile([P, FD], f32, tag="tdiv")
                nc.vector.tensor_tensor(tdiv[:], yv, trc[:], ALU.mult)
                ta = tp.tile([P, FD], f32, tag="ta")
                nc.scalar.activation(ta[:], tdiv[:], AFT.Arctan)
                ts = tp.tile([P, FD], f32, tag="ts")
                nc.scalar.activation(ts[:], zv, AFT.Sign)
                tu = tp.tile([P, FD], f32, tag="tu")
                nc.vector.scalar_tensor_tensor(
                    tu[:], ts[:], -math.pi / 2.0, ta[:], ALU.mult, ALU.add)
                nc.scalar.activation(ov1, tu[:], AFT.Copy, scale=-FACTOR)

                # out0 = -90*clip(x,-1,1) - 90
                tclip = tp.tile([P, FD], f32, tag="tclip")
                nc.vector.tensor_scalar(
                    tclip[:], xv, 1.0, -1.0, ALU.min, ALU.max)
                nc.scalar.activation(
                    ov0, tclip[:], AFT.Copy, bias=-90.0, scale=-90.0)

                nc.sync.dma_start(ot[t], tout[:])
    nc.compile()
    return nc


def _get_nc():
    if 'nc' not in _CACHE:
        _CACHE['nc'] = _build()
    return _CACHE['nc']


def kernel(inputs):
    from concourse import bass_utils
    inputs = np.ascontiguousarray(inputs, dtype=np.float32)
    assert inputs.shape == (B, L, 3), inputs.shape
    nc = _get_nc()
    in_maps = [
        {"x": inputs[c * BPC:(c + 1) * BPC].reshape(-1)} for c in range(NCORES)
    ]
    res = bass_utils.run_bass_kernel_spmd(nc, in_maps, list(range(NCORES)))
    out = np.concatenate(
        [res.results[c]["o"].reshape(BPC, L, 2) for c in range(NCORES)], axis=0)
    return out


# revision 15
# speedup vs baseline: 1.0428x; 1.0068x over previous
"""Trainium2 Bass kernel for nn_CilLayer: [128,65536,3] f32 -> [128,65536,2] f32.

out0 = -90*(clip(x,-1,1)+1)
out1 = (180/pi)*atan2(z,y)  computed as  180*(z>=0) - 90 - (180/pi)*atan(y/z)

Sharding: batch dim split evenly across 8 NeuronCores (16 batches/core),
purely elementwise, no communication.
"""
import sys
import math

if '/opt/trn_rl_repo' not in sys.path:
    sys.path.insert(0, '/opt/trn_rl_repo')

import numpy as np

B, L = 128, 65536
NCORES = 8
BPC = B // NCORES            # batches per core
NPT = BPC * L                # points per core = 1,048,576
P = 128                      # SBUF partitions
FACTOR = 180.0 / math.pi

_CACHE = {}


def _build():
    from concourse import mybir, tile, bacc
    f32 = mybir.dt.float32
    AFT = mybir.ActivationFunctionType
    ALU = mybir.AluOpType

    nc = bacc.Bacc("TRN2", debug=False)
    x = nc.dram_tensor("x", [NPT * 3], f32, kind="ExternalInput").ap()
    o = nc.dram_tensor("o", [NPT * 2], f32, kind="ExternalOutput").ap()

    # per-partition point counts per tile: small edge tiles to shorten
    # pipeline ramp and drain, big tiles in the middle
    chunks = [256, 256, 512] + [1024] * 6 + [512, 256, 256]
    assert sum(chunks) == NPT // P

    with tile.TileContext(nc) as tc:
        with tc.tile_pool(name="io", bufs=4) as iop, \
             tc.tile_pool(name="tmp", bufs=2) as tp:
            off = 0  # running offset in points
            for fd in chunks:
                xin_ap = x[off * 3:(off + P * fd) * 3].rearrange(
                    "(p m) -> p m", p=P)
                oout_ap = o[off * 2:(off + P * fd) * 2].rearrange(
                    "(p m) -> p m", p=P)
                off += P * fd
                tin = iop.tile([P, 3 * fd], f32, tag="in")
                nc.sync.dma_start(tin[:], xin_ap)
                v = tin[:].rearrange("p (f c) -> p f c", c=3)
                xv, yv, zv = v[:, :, 0], v[:, :, 1], v[:, :, 2]

                tout = iop.tile([P, 2 * fd], f32, tag="out")
                ov = tout[:].rearrange("p (f c) -> p f c", c=2)
                ov0, ov1 = ov[:, :, 0], ov[:, :, 1]

                # out1 = -FACTOR*(atan(y/z) - (pi/2)*sign(z))
                trc = tp.tile([P, fd], f32, tag="trc")
                nc.vector.reciprocal_approx_fast(trc[:], zv)
                tdiv = tp.tile([P, fd], f32, tag="tdiv")
                nc.vector.tensor_tensor(tdiv[:], yv, trc[:], ALU.mult)
                ta = tp.tile([P, fd], f32, tag="ta")
                nc.scalar.activation(ta[:], tdiv[:], AFT.Arctan)
                ts = tp.tile([P, fd], f32, tag="ts")
                nc.scalar.activation(ts[:], zv, AFT.Sign)
                tu = tp.tile([P, fd], f32, tag="tu")
                nc.vector.scalar_tensor_tensor(
                    tu[:], ts[:], -math.pi / 2.0, ta[:], ALU.mult, ALU.add)
                nc.scalar.activation(ov1, tu[:], AFT.Copy, scale=-FACTOR)

                # out0 = -90*clip(x,-1,1) - 90
                tclip = tp.tile([P, fd], f32, tag="tclip")
                nc.vector.tensor_scalar(
                    tclip[:], xv, 1.0, -1.0, ALU.min, ALU.max)
                nc.scalar.activation(
                    ov0, tclip[:], AFT.Copy, bias=-90.0, scale=-90.0)

                nc.sync.dma_start(oout_ap, tout[:])
    nc.compile()
    return nc


def _get_nc():
    if 'nc' not in _CACHE:
        _CACHE['nc'] = _build()
    return _CACHE['nc']


def kernel(inputs):
    from concourse import bass_utils
    inputs = np.ascontiguousarray(inputs, dtype=np.float32)
    assert inputs.shape == (B, L, 3), inputs.shape
    nc = _get_nc()
    in_maps = [
        {"x": inputs[c * BPC:(c + 1) * BPC].reshape(-1)} for c in range(NCORES)
    ]
    res = bass_utils.run_bass_kernel_spmd(nc, in_maps, list(range(NCORES)))
    out = np.concatenate(
        [res.results[c]["o"].reshape(BPC, L, 2) for c in range(NCORES)], axis=0)
    return out


# revision 16
# speedup vs baseline: 1.1879x; 1.1391x over previous
"""Trainium2 Bass kernel for nn_CilLayer: [128,65536,3] f32 -> [128,65536,2] f32.

out0 = -90*(clip(x,-1,1)+1)
out1 = (180/pi)*atan2(z,y)  computed as  180*(z>=0) - 90 - (180/pi)*atan(y/z)

Sharding: batch dim split evenly across 8 NeuronCores (16 batches/core),
purely elementwise, no communication.
"""
import sys
import math

if '/opt/trn_rl_repo' not in sys.path:
    sys.path.insert(0, '/opt/trn_rl_repo')

import numpy as np

B, L = 128, 65536
NCORES = 8
BPC = B // NCORES            # batches per core
NPT = BPC * L                # points per core = 1,048,576
P = 128                      # SBUF partitions
FACTOR = 180.0 / math.pi

_CACHE = {}


def _build():
    from concourse import mybir, tile, bacc
    f32 = mybir.dt.float32
    AFT = mybir.ActivationFunctionType
    ALU = mybir.AluOpType

    nc = bacc.Bacc("TRN2", debug=False)
    x = nc.dram_tensor("x", [NPT * 3], f32, kind="ExternalInput").ap()
    o = nc.dram_tensor("o", [NPT * 2], f32, kind="ExternalOutput").ap()

    # per-partition point counts per tile: small edge tiles to shorten
    # pipeline ramp and drain, big tiles in the middle
    chunks = [256, 256, 512] + [1024] * 6 + [512, 256, 256]
    assert sum(chunks) == NPT // P

    with tile.TileContext(nc) as tc:
        with tc.tile_pool(name="io", bufs=4) as iop, \
             tc.tile_pool(name="tmp", bufs=2) as tp:
            off = 0  # running offset in points
            for fd in chunks:
                xin_ap = x[off * 3:(off + P * fd) * 3].rearrange(
                    "(p m) -> p m", p=P)
                oout_ap = o[off * 2:(off + P * fd) * 2].rearrange(
                    "(p m) -> p m", p=P)
                off += P * fd
                tin = iop.tile([P, 3 * fd], f32, tag="in")
                nc.sync.dma_start(tin[:], xin_ap)
                v = tin[:].rearrange("p (f c) -> p f c", c=3)
                xv, yv, zv = v[:, :, 0], v[:, :, 1], v[:, :, 2]

                tout = iop.tile([P, 2 * fd], f32, tag="out")
                ov = tout[:].rearrange("p (f c) -> p f c", c=2)
                ov0, ov1 = ov[:, :, 0], ov[:, :, 1]

                # out1 = -FACTOR*(atan(y/z) - (pi/2)*sign(z))
                trc = tp.tile([P, fd], f32, tag="trc")
                nc.vector.reciprocal_approx_fast(trc[:], zv)
                tdiv = tp.tile([P, fd], f32, tag="tdiv")
                nc.vector.tensor_tensor(tdiv[:], yv, trc[:], ALU.mult)
                ta = tp.tile([P, fd], f32, tag="ta")
                nc.scalar.activation(ta[:], tdiv[:], AFT.Arctan)
                ts = tp.tile([P, fd], f32, tag="ts")
                nc.scalar.activation(ts[:], zv, AFT.Sign)
                tu = tp.tile([P, fd], f32, tag="tu")
                nc.vector.scalar_tensor_tensor(
                    tu[:], ts[:], -math.pi / 2.0, ta[:], ALU.mult, ALU.add)
                nc.scalar.activation(ov1, tu[:], AFT.Copy, scale=-FACTOR)

                # out0 = -90*clip(x,-1,1) - 90
                tclip = tp.tile([P, fd], f32, tag="tclip")
                nc.vector.tensor_scalar(
                    tclip[:], xv, 1.0, -1.0, ALU.min, ALU.max)
                nc.scalar.activation(
                    ov0, tclip[:], AFT.Copy, bias=-90.0, scale=-90.0)

                nc.gpsimd.dma_start(oout_ap, tout[:])
    nc.compile()
    return nc


def _get_nc():
    if 'nc' not in _CACHE:
        _CACHE['nc'] = _build()
    return _CACHE['nc']


def kernel(inputs):
    from concourse import bass_utils
    inputs = np.ascontiguousarray(inputs, dtype=np.float32)
    assert inputs.shape == (B, L, 3), inputs.shape
    nc = _get_nc()
    in_maps = [
        {"x": inputs[c * BPC:(c + 1) * BPC].reshape(-1)} for c in range(NCORES)
    ]
    res = bass_utils.run_bass_kernel_spmd(nc, in_maps, list(range(NCORES)))
    out = np.concatenate(
        [res.results[c]["o"].reshape(BPC, L, 2) for c in range(NCORES)], axis=0)
    return out


# revision 18
# speedup vs baseline: 1.1883x; 1.0004x over previous
"""Trainium2 Bass kernel for nn_CilLayer: [128,65536,3] f32 -> [128,65536,2] f32.

out0 = -90*(clip(x,-1,1)+1)
out1 = (180/pi)*atan2(z,y)  computed as  180*(z>=0) - 90 - (180/pi)*atan(y/z)

Sharding: batch dim split evenly across 8 NeuronCores (16 batches/core),
purely elementwise, no communication.
"""
import sys
import math

if '/opt/trn_rl_repo' not in sys.path:
    sys.path.insert(0, '/opt/trn_rl_repo')

import numpy as np

B, L = 128, 65536
NCORES = 8
BPC = B // NCORES            # batches per core
NPT = BPC * L                # points per core = 1,048,576
P = 128                      # SBUF partitions
FACTOR = 180.0 / math.pi

_CACHE = {}


def _build():
    from concourse import mybir, tile, bacc
    f32 = mybir.dt.float32
    AFT = mybir.ActivationFunctionType
    ALU = mybir.AluOpType

    nc = bacc.Bacc("TRN2", debug=False)
    x = nc.dram_tensor("x", [NPT * 3], f32, kind="ExternalInput").ap()
    o = nc.dram_tensor("o", [NPT * 2], f32, kind="ExternalOutput").ap()

    # per-partition point counts per tile: small edge tiles to shorten
    # pipeline ramp and drain, big tiles in the middle
    chunks = [256, 256, 512] + [1024] * 6 + [512, 256, 256]
    assert sum(chunks) == NPT // P

    with tile.TileContext(nc) as tc:
        with tc.tile_pool(name="io", bufs=4) as iop, \
             tc.tile_pool(name="tmp", bufs=2) as tp:
            off = 0  # running offset in points
            for ci, fd in enumerate(chunks):
                tail = ci >= len(chunks) - 3
                xin_ap = x[off * 3:(off + P * fd) * 3].rearrange(
                    "(p m) -> p m", p=P)
                oout_ap = o[off * 2:(off + P * fd) * 2].rearrange(
                    "(p m) -> p m", p=P)
                off += P * fd
                tin = iop.tile([P, 3 * fd], f32, tag="in")
                nc.sync.dma_start(tin[:], xin_ap)
                v = tin[:].rearrange("p (f c) -> p f c", c=3)
                xv, yv, zv = v[:, :, 0], v[:, :, 1], v[:, :, 2]

                tout = iop.tile([P, 2 * fd], f32, tag="out")
                ov = tout[:].rearrange("p (f c) -> p f c", c=2)
                ov0, ov1 = ov[:, :, 0], ov[:, :, 1]

                # out1 = -FACTOR*(atan(y/z) - (pi/2)*sign(z))
                trc = tp.tile([P, fd], f32, tag="trc")
                nc.vector.reciprocal_approx_fast(trc[:], zv)
                tdiv = tp.tile([P, fd], f32, tag="tdiv")
                nc.vector.tensor_tensor(tdiv[:], yv, trc[:], ALU.mult)
                ta = tp.tile([P, fd], f32, tag="ta")
                nc.scalar.activation(ta[:], tdiv[:], AFT.Arctan)
                ts = tp.tile([P, fd], f32, tag="ts")
                nc.scalar.activation(ts[:], zv, AFT.Sign)
                tu = tp.tile([P, fd], f32, tag="tu")
                nc.vector.scalar_tensor_tensor(
                    tu[:], ts[:], -math.pi / 2.0, ta[:], ALU.mult, ALU.add)
                if tail:
                    nc.vector.tensor_scalar(
                        ov1, tu[:], -FACTOR, None, ALU.mult)
                else:
                    nc.scalar.activation(
                        ov1, tu[:], AFT.Copy, scale=-FACTOR)

                # out0 = -90*clip(x,-1,1) - 90
                tclip = tp.tile([P, fd], f32, tag="tclip")
                nc.vector.tensor_scalar(
                    tclip[:], xv, 1.0, -1.0, ALU.min, ALU.max)
                if tail:
                    nc.vector.tensor_scalar(
                        ov0, tclip[:], -90.0, -90.0, ALU.mult, ALU.add)
                else:
                    nc.scalar.activation(
                        ov0, tclip[:], AFT.Copy, bias=-90.0, scale=-90.0)

                nc.gpsimd.dma_start(oout_ap, tout[:])
    nc.compile()
    return nc


def _get_nc():
    if 'nc' not in _CACHE:
        _CACHE['nc'] = _build()
    return _CACHE['nc']


def kernel(inputs):
    from concourse import bass_utils
    inputs = np.ascontiguousarray(inputs, dtype=np.float32)
    assert inputs.shape == (B, L, 3), inputs.shape
    nc = _get_nc()
    in_maps = [
        {"x": inputs[c * BPC:(c + 1) * BPC].reshape(-1)} for c in range(NCORES)
    ]
    res = bass_utils.run_bass_kernel_spmd(nc, in_maps, list(range(NCORES)))
    out = np.concatenate(
        [res.results[c]["o"].reshape(BPC, L, 2) for c in range(NCORES)], axis=0)
    return out
